# revision 33
# baseline (speedup 1.0000x reference)
"""3-layer GCN forward on 8 TRN2 NeuronCores (Bass/Tile).

Per layer: out = dinv*(A @ T + T) @ W + b, with T = dinv*h, dinv=1/sqrt(deg+1),
A = binary adjacency (dupes count, no self loops), leaky_relu(0.2) between.

Device plan (per core, dst-sharded in blocks):
- per-layer gather tables [8*SR,128] / [8*SB,128] bf16 rows of T (cols 64..
  unused), split by source-node block-row half so the inter-layer AllGather
  pipelines: AG-A (rows [0,SR) of every block) unblocks half-0 edge classes
  while AG-B is still on the wire.
- per-edge dma_gather (256B rows, int16 idx); 4 token classes =
  (src A/B half for int16 range) x (row half for AG pipelining).
- segment sums on PE with constant per-degree block-ones matrices (nodes
  binned by class in-degree, 128-token planes, k0 levels 0/32/64 + a 96
  level via zero-padded accumulating matmul);
  node sums dma_scatter_add'ed (unique idx per call, 128B bf16 payloads at
  256B stride) into per-core AGG DRAM.
- epilogue per 128-row tile: Z=(AGG_A+AGG_B+T)*dinv; PE transpose; @W; +b;
  Lrelu; *dinv -> cc_in; split AllGather -> next tables.
"""
import numpy as np

NEG_SLOPE = 0.2
_TIMING = False  # strip custom-DMA sems so TimelineSim can run
_SKIP_CC = False




class _Cfg:
    def __init__(self, n_nodes, d_in=64, d_out=4, ch_planes=64, sc_cols=32,
                 ntrash=4352):
        self.W = 8
        self.N = n_nodes
        self.D = d_in
        self.DOUT = d_out
        self.BLK = (n_nodes + self.W - 1) // self.W
        self.BP = ((self.BLK + 127) // 128) * 128
        self.NT = self.W * self.BP
        self.SRC_SPLIT = (self.W // 2) * self.BLK
        self.NTRASH = ntrash
        self.CH_PLANES = ch_planes
        self.SC_COLS = sc_cols
        self.NTILES = self.BP // 128
        self.EG = 7 if self.NTILES % 7 == 0 else 1
        self.NGRP = self.NTILES // self.EG
        # segment the block rows at epilogue-group granularity so the
        # inter-layer AllGather splits into pipelined pieces
        self.NSEG = 3 if self.NGRP >= 3 else self.NGRP
        gb = [j * self.NGRP // self.NSEG for j in range(self.NSEG)]
        gb.append(self.NGRP)
        self.GB = gb                            # group boundaries
        rb = [g * self.EG * 128 for g in gb[:-1]] + [self.BP]
        self.RB = rb                            # block-row boundaries
        self.SEGR = [rb[j + 1] - rb[j] for j in range(self.NSEG)]
        self.CLS = [(side, seg) for seg in range(self.NSEG)
                    for side in (0, 1)]


def _pack_side(cfg, node_counts_max, per_core_nodes, per_core_srcs, side,
               seg):
    """Shared schedule + per-core token/slot data for one edge class."""
    W = cfg.W
    blocks = []
    plane_cursor = 0
    col_cursor = 0
    bins = sorted(d for d, n in node_counts_max.items() if d > 0 and n > 0)
    for d in bins:
        n = node_counts_max[d]
        spp = 128 // d
        assert spp >= 1, f"degree {d} > 128 unsupported"
        nq = (spp + 31) // 32
        span = 32 * nq
        if d == 1:
            nlev = 1
        elif spp <= 32:
            # bases 0/32/64 direct; the 4th level (k0=96) is emitted as a
            # zero-padded accumulating matmul at base 64
            nlev = 4
        else:
            nlev = max(1, 128 // span)
        lev_k0 = ([lev * 32 for lev in range(nlev)] if spp <= 32
                  else [lev * span for lev in range(nlev)])
        ncols_max = max(1, min(8, cfg.CH_PLANES // nlev))
        P = (n + spp - 1) // spp
        p = 0
        while p < P:
            bplanes = min(ncols_max * nlev, P - p)
            ncols = min(ncols_max, bplanes)
            mms = []
            lev, q = 0, 0
            while q < bplanes:
                nb = min(ncols, bplanes - q)
                mms.append(dict(k0=lev_k0[lev], nb=nb,
                                plane0=plane_cursor + q))
                q += nb
                lev += 1
            blocks.append(dict(col0=col_cursor, ncols=ncols, matmuls=mms,
                               plane0=plane_cursor, planes=bplanes, d=d,
                               spp=spp))
            plane_cursor += bplanes
            col_cursor += ncols
            p += bplanes
    ntok = plane_cursor * 128
    ncols_total = col_cursor

    # plane -> (d, first node slot) in stream order
    plane_slots = [None] * plane_cursor
    cur = {d: 0 for d in bins}
    for blk in blocks:
        d, spp = blk['d'], blk['spp']
        for mm in blk['matmuls']:
            for c in range(mm['nb']):
                plane_slots[mm['plane0'] + c] = (d, cur[d])
                cur[d] += spp

    # gather row index within the (side, seg) table; holes use row 0
    # (any valid row: hole tokens only feed hole slots -> trash agg rows)
    rh = cfg.SEGR[seg]
    gidx, sidx = [], []
    for r in range(W):
        tok = np.full(ntok, -1, np.int64)
        for pidx in range(plane_cursor):
            d, s0 = plane_slots[pidx]
            base = pidx * 128
            nodes = per_core_nodes[r].get(d, [])
            srcs = per_core_srcs[r].get(d, [])
            spp = 128 // d
            for k in range(spp):
                slot = s0 + k
                if slot < len(nodes):
                    tok[base + k * d: base + (k + 1) * d] = srcs[slot]
        loc = tok % cfg.BLK - cfg.RB[seg]
        rows = np.where(
            tok >= 0,
            (tok // cfg.BLK) * rh + loc
            - ((cfg.W // 2) * rh if side == 1 else 0),
            0)
        if ntok:
            assert rows.min() >= 0 and rows.max() < 32768, \
                (rows.min(), rows.max())
        gidx.append(rows.astype(np.int16))

        nsc = ncols_total * 128
        sl = np.full(nsc, -1, np.int64)
        for blk in blocks:
            d, spp = blk['d'], blk['spp']
            nodes = per_core_nodes[r].get(d, [])
            for mm in blk['matmuls']:
                for c in range(mm['nb']):
                    _, s0 = plane_slots[mm['plane0'] + c]
                    col = blk['col0'] + c
                    for k in range(spp):
                        if s0 + k < len(nodes):
                            sl[col * 128 + mm['k0'] + k] = nodes[s0 + k]
        ncalls = (ncols_total + cfg.SC_COLS - 1) // cfg.SC_COLS
        for call in range(ncalls):
            lo = call * cfg.SC_COLS * 128
            hi = min((call + 1) * cfg.SC_COLS * 128, nsc)
            hole = np.where(sl[lo:hi] < 0)[0]
            assert len(hole) <= cfg.NTRASH, f"trash overflow {len(hole)}"
            sl[lo + hole] = cfg.BP + np.arange(len(hole))
        sidx.append(sl.astype(np.int16))

    chunks = []
    cur_c = None
    for bi, blk in enumerate(blocks):
        if cur_c is None or cur_c['planes'] + blk['planes'] > cfg.CH_PLANES:
            cur_c = dict(plane0=blk['plane0'], planes=0, blocks=[])
            chunks.append(cur_c)
        cur_c['planes'] += blk['planes']
        cur_c['blocks'].append(bi)
    return (dict(blocks=blocks, chunks=chunks, ncols=ncols_total, ntok=ntok),
            gidx, sidx)


def _preprocess(cfg, edge_index):
    W, N, BLK = cfg.W, cfg.N, cfg.BLK
    src = np.asarray(edge_index[0], np.int64)
    dst = np.asarray(edge_index[1], np.int64)
    deg = np.bincount(dst, minlength=N).astype(np.float64) + 1.0
    dinv = (1.0 / np.sqrt(deg)).astype(np.float32)

    classes = []
    for side, seg in cfg.CLS:
        mask_side = (src >= cfg.SRC_SPLIT) == (side == 1)
        lo_r, hi_r = cfg.RB[seg], cfg.RB[seg + 1]
        loc = src % BLK
        mask = mask_side & (loc >= lo_r) & (loc < hi_r)
        ncmax = {}
        pc_nodes, pc_srcs = [], []
        for r in range(W):
            lo, hi = r * BLK, min((r + 1) * BLK, N)
            m = mask & (dst >= lo) & (dst < hi)
            s_r, v_r = src[m], dst[m] - lo
            order = np.argsort(v_r, kind='stable')
            s_r, v_r = s_r[order], v_r[order]
            nodes, starts, counts = np.unique(
                v_r, return_index=True, return_counts=True)
            bn, bs = {}, {}
            for j in range(len(nodes)):
                d = int(counts[j])
                bn.setdefault(d, []).append(int(nodes[j]))
                bs.setdefault(d, []).append(s_r[starts[j]:starts[j] + d])
            pc_nodes.append(bn)
            pc_srcs.append(bs)
            for d, lst in bn.items():
                ncmax[d] = max(ncmax.get(d, 0), len(lst))
        classes.append(_pack_side(cfg, ncmax, pc_nodes, pc_srcs, side, seg))
    return dinv, classes


def _wrap16(a):
    a = np.asarray(a, np.int16)
    assert a.size % 16 == 0
    w = np.ascontiguousarray(a.reshape(-1, 16).T)
    return np.tile(w, (8, 1))


def _build(cfg, sides, s_offsets, z_offsets, s_total):
    import concourse.bacc as bacc
    import concourse.mybir as mybir
    import concourse.tile as tile
    import concourse.masks as masks

    D, DOUT, BP, W = cfg.D, cfg.DOUT, cfg.BP, cfg.W
    NSEG, SEGR, RB = cfg.NSEG, cfg.SEGR, cfg.RB
    NCLS = len(cfg.CLS)
    NTILES = cfg.NTILES
    f32, bf16, i16 = mybir.dt.float32, mybir.dt.bfloat16, mybir.dt.int16

    nc = bacc.Bacc(None, target_bir_lowering=False)
    tbl0 = [nc.dram_tensor(f"tbl0s{j}", [W * SEGR[j], 128], bf16,
                           kind="ExternalInput") for j in range(NSEG)]
    t0_blk = nc.dram_tensor("t0_blk", [BP, D], bf16, kind="ExternalInput")
    dinv_blk = nc.dram_tensor("dinv_blk", [128, NTILES], f32, kind="ExternalInput")
    w0 = nc.dram_tensor("w0", [D, D], f32, kind="ExternalInput")
    w1 = nc.dram_tensor("w1", [D, D], f32, kind="ExternalInput")
    w2 = nc.dram_tensor("w2", [D, DOUT], f32, kind="ExternalInput")
    b01 = nc.dram_tensor("b01", [128, 2 * D], f32, kind="ExternalInput")
    b2b = nc.dram_tensor("b2b", [128, DOUT], f32, kind="ExternalInput")
    smat = nc.dram_tensor("smat", [128, s_total], bf16, kind="ExternalInput")
    gidx_in = [nc.dram_tensor(f"gidx{c}", [128, sides[c][0]['ntok'] // 16],
                              i16, kind="ExternalInput")
               if sides[c][0]['ntok'] else None for c in range(NCLS)]
    sidx_in = [nc.dram_tensor(f"sidx{c}", [128, sides[c][0]['ncols'] * 8],
                              i16, kind="ExternalInput")
               if sides[c][0]['ncols'] else None for c in range(NCLS)]
    outr = nc.dram_tensor("outr", [BP, DOUT], f32, kind="ExternalOutput")

    cc_in = [nc.dram_tensor(f"cc_in{j}", [SEGR[j], D], bf16)
             for j in range(NSEG)]
    cc_out = [nc.dram_tensor(f"cc_out{j}", [W * SEGR[j], D], bf16,
                             addr_space="Shared") for j in range(NSEG)]
    tbl_cc = [nc.dram_tensor(f"tbl_cc{j}", [W * SEGR[j], 128], bf16)
              for j in range(NSEG)]
    agg = nc.dram_tensor("agg", [BP + cfg.NTRASH, 128], bf16)

    with tile.TileContext(nc) as tc:
        with (
            tc.tile_pool(name="const", bufs=1) as cpool,
            tc.tile_pool(name="msg", bufs=3) as msgpool,
            tc.tile_pool(name="work", bufs=2) as work,
            tc.tile_pool(name="epi", bufs=3) as epi,
            tc.tile_pool(name="psum", bufs=3, space="PSUM") as psum,
            tc.tile_pool(name="psum_e", bufs=2, space="PSUM") as psum_e,
        ):
            gsem = [nc.alloc_semaphore(f"gsem{i}") for i in range(4)]
            gcnt = [0] * 4
            ssem = [[nc.alloc_semaphore(f"ssem{i}{s}") for s in (0, 1)]
                    for i in range(3)]
            scnt = [[0, 0] for _ in range(3)]

            ident = cpool.tile([128, 128], f32)
            masks.make_identity(nc, ident[:])
            w0t = cpool.tile([D, D], f32)
            w1t = cpool.tile([D, D], f32)
            w2t = cpool.tile([D, DOUT], f32)
            nc.sync.dma_start(w0t[:], w0[:])
            nc.sync.dma_start(w1t[:], w1[:])
            nc.sync.dma_start(w2t[:], w2[:])
            b01t = cpool.tile([128, 2 * D], f32)
            nc.sync.dma_start(b01t[:], b01[:])
            b2t = cpool.tile([128, DOUT], f32)
            nc.sync.dma_start(b2t[:], b2b[:])
            dinv_blk_t = cpool.tile([128, NTILES], f32)
            nc.sync.dma_start(dinv_blk_t[:], dinv_blk[:])
            smat_t = cpool.tile([128, s_total], bf16)
            nc.sync.dma_start(smat_t[:], smat[:])
            sixt, gixt, stage = [], [], []
            for c in range(NCLS):
                if sides[c][0]['ncols']:
                    st = cpool.tile([128, sides[c][0]['ncols'] * 8], i16,
                                    tag=f"six{c}", name=f"sixt{c}")
                    nc.sync.dma_start(st[:], sidx_in[c][:])
                    sixt.append(st)
                    sg = cpool.tile([128, sides[c][0]['ncols'], D], bf16,
                                    tag=f"stage{c}", name=f"stage{c}")
                    nc.vector.memset(sg[:], 0.0)
                    stage.append(sg)
                else:
                    sixt.append(None)
                    stage.append(None)
                if sides[c][0]['ntok']:
                    gt = cpool.tile([128, sides[c][0]['ntok'] // 16], i16,
                                    tag=f"gix{c}", name=f"gixt{c}")
                    nc.sync.dma_start(gt[:], gidx_in[c][:])
                    gixt.append(gt)
                else:
                    gixt.append(None)

            ZW = 2048
            ztile = cpool.tile([128, ZW], bf16, tag="zero")
            nc.vector.memset(ztile[:], 0.0)

            def zero_dram(t_ap):
                """Zero a bf16 DRAM region via flat chunks."""
                flat = t_ap.rearrange("(p a) c -> p (a c)", p=128)
                n = flat.shape[1]
                off = 0
                while off < n:
                    m = min(ZW, n - off)
                    nc.sync.dma_start(flat[:, off:off + m], ztile[:, :m])
                    off += m

            for layer in range(3):
                tabs = tbl0 if layer == 0 else tbl_cc

                # zero AGG (Tile orders scatters after via WAW on agg)
                zero_dram(agg[0:BP, :])

                def table_copy(dst, srcT, rows):
                    CH = 14336  # rows per copy call (desc limit 16384)
                    for r0 in range(0, rows, CH):
                        r1 = min(rows, r0 + CH)
                        nc.sync.dma_start(
                            dst[r0:r1, 0:D].rearrange(
                                "(p a) c -> p a c", p=128),
                            srcT[r0:r1, :].rearrange(
                                "(p a) c -> p a c", p=128))

                for c in range(NCLS):
                    side, seg = cfg.CLS[c]
                    # stage the seg-table copy just before its first
                    # consumer class: Pool stays free for earlier segs'
                    # gathers while later AG pieces are still on the wire
                    if layer > 0 and not _SKIP_CC and c == 2 * seg:
                        table_copy(tbl_cc[seg], cc_out[seg], W * SEGR[seg])
                    sched = sides[c][0]
                    if not sched['ntok']:
                        continue
                    tab = tabs[seg]
                    rh = SEGR[seg]
                    base = tab[:, :] if side == 0 else tab[(W // 2) * rh:, :]
                    blocks = sched['blocks']
                    for ci, ch in enumerate(sched['chunks']):
                        gx = gixt[c][:, ch['plane0'] * 8:
                                     (ch['plane0'] + ch['planes']) * 8]
                        msg = msgpool.tile([128, ch['planes'], 128], bf16,
                                           tag="msg")
                        k = (layer * 5 + c * 3 + ci) % 4
                        g = nc.gpsimd.dma_gather(
                            msg[:], base, gx, ch['planes'] * 128,
                            ch['planes'] * 128, 128, single_packet=False)
                        if not _TIMING:
                            g.then_inc(gsem[k], 16)
                        gcnt[k] += 16
                        gthresh = gcnt[k]
                        for bi in ch['blocks']:
                            blk = blocks[bi]
                            if blk['d'] == 1:
                                # degree-1: token IS the sum; copy via temp
                                for mm in blk['matmuls']:
                                    p0 = mm['plane0'] - ch['plane0']
                                    tmp = work.tile([128, 8, D], f32,
                                                    tag="d1tmp")
                                    c1 = nc.vector.tensor_copy(
                                        tmp[:, 0:mm['nb'], :],
                                        msg[:, p0:p0 + mm['nb'], 0:D])
                                    if not _TIMING:
                                        c1._wait_ge(gsem[k], gthresh)
                                    c2 = nc.scalar.copy(
                                        stage[c][:, blk['col0']:
                                                 blk['col0'] + mm['nb'], :],
                                        tmp[:, 0:mm['nb'], :])
                                    if layer > 0 and not _TIMING:
                                        c2._wait_ge(ssem[layer - 1][side],
                                                    scnt[layer - 1][side])
                                continue
                            soff = s_offsets[blk['d']]
                            pt = psum.tile([128, 8, D], f32, tag="segsum")
                            spp = blk['spp']
                            has96 = any(mm['k0'] == 96
                                        for mm in blk['matmuls'])
                            for mm in blk['matmuls']:
                                p0 = mm['plane0'] - ch['plane0']
                                if mm['k0'] == 96:
                                    # accumulate onto the zero-padded tail of
                                    # the base-64 level (out base 0/32/64)
                                    zoff = z_offsets[blk['d']]
                                    m = nc.tensor.matmul(
                                        pt[64:96 + spp, 0:mm['nb'], :],
                                        smat_t[:, zoff + spp:
                                               zoff + 2 * spp + 32],
                                        msg[:, p0:p0 + mm['nb'], 0:D],
                                        start=False, stop=True,
                                        skip_group_check=True)
                                elif mm['k0'] == 64 and has96:
                                    # wide write: zeros over the lev3 range
                                    zoff = z_offsets[blk['d']]
                                    m = nc.tensor.matmul(
                                        pt[64:96 + spp, 0:mm['nb'], :],
                                        smat_t[:, zoff:zoff + spp + 32],
                                        msg[:, p0:p0 + mm['nb'], 0:D],
                                        start=True, stop=False,
                                        skip_group_check=True)
                                else:
                                    m = nc.tensor.matmul(
                                        pt[mm['k0']:mm['k0'] + spp,
                                           0:mm['nb'], :],
                                        smat_t[:, soff:soff + spp],
                                        msg[:, p0:p0 + mm['nb'], 0:D])
                                if not _TIMING:
                                    m._wait_ge(gsem[k], gthresh)
                            copies = [
                                (slice(mm['k0'], mm['k0'] + blk['spp']),
                                 slice(0, mm['nb']))
                                for mm in blk['matmuls']]
                            for cj, (kr, cr) in enumerate(copies):
                                dst = stage[c][kr, blk['col0'] + cr.start:
                                               blk['col0'] + cr.stop, :]
                                srcp = pt[kr, cr, :]
                                if (bi + cj) % 2 == 0:
                                    c_ins = nc.vector.tensor_copy(dst, srcp)
                                else:
                                    c_ins = nc.scalar.copy(dst, srcp)
                                if layer > 0 and not _TIMING:
                                    c_ins._wait_ge(ssem[layer - 1][side],
                                                   scnt[layer - 1][side])
                    nco = sched['ncols']
                    ncall = (nco + cfg.SC_COLS - 1) // cfg.SC_COLS
                    # calls within a class hit disjoint agg rows (one slot
                    # per node; trash rows are garbage) -> only serialize
                    # against PRIOR same-side classes' RMW traffic
                    cls_thresh = scnt[layer][side]
                    for call in range(ncall):
                        c0 = call * cfg.SC_COLS
                        ncc = min(cfg.SC_COLS, nco - c0)
                        sc = nc.gpsimd.dma_scatter_add(
                            agg[:, side * D:(side + 1) * D],
                            stage[c][:, c0:c0 + ncc, :],
                            sixt[c][:, c0 * 8:(c0 + ncc) * 8],
                            ncc * 128, ncc * 128, D, elem_step=128,
                            single_packet=False)
                        if cls_thresh and not _TIMING:
                            sc._wait_ge(ssem[layer][side], cls_thresh)
                        if not _TIMING:
                            sc.then_inc(ssem[layer][side], 16)
                        scnt[layer][side] += 16

                sc_thresh = (scnt[layer][0], scnt[layer][1])
                # epilogue, batched in groups of EG row-tiles.
                # Row mapping within a group: row = g*EG*128 + p*EG + a
                # (partition p, slice a); dinv_blk input uses same layout.
                EG = cfg.EG
                NGRP = cfg.NGRP
                gseg = [max(j for j in range(cfg.NSEG) if cfg.GB[j] <= g)
                        for g in range(NGRP)]
                for g in range(NGRP):
                    r0, r1 = g * EG * 128, (g + 1) * EG * 128
                    sj = gseg[g]
                    sr0, sr1 = r0 - RB[sj], r1 - RB[sj]
                    t_src = (t0_blk[r0:r1, :] if layer == 0
                             else cc_in[sj][sr0:sr1, :])
                    a0 = epi.tile([128, EG, D], bf16, tag="a0")
                    a1 = epi.tile([128, EG, D], bf16, tag="a1")
                    d0 = nc.sync.dma_start(
                        a0[:], agg[r0:r1, 0:D].rearrange(
                            "(p a) c -> p a c", p=128))
                    d1 = nc.sync.dma_start(
                        a1[:], agg[r0:r1, D:2 * D].rearrange(
                            "(p a) c -> p a c", p=128))
                    if not _TIMING:
                        d0._wait_ge(ssem[layer][0], sc_thresh[0])
                        d1._wait_ge(ssem[layer][1], sc_thresh[1])
                    tb = epi.tile([128, EG, D], bf16, tag="tb")
                    nc.sync.dma_start(
                        tb[:], t_src.rearrange("(p a) c -> p a c", p=128))
                    tf = epi.tile([128, EG, D], f32, tag="tf")
                    nc.vector.tensor_copy(tf[:], tb[:])
                    z = epi.tile([128, EG, D], f32, tag="z")
                    nc.vector.tensor_add(z[:], a0[:], a1[:])
                    nc.vector.tensor_add(z[:], z[:], tf[:])
                    if layer < 2:
                        ot = epi.tile([128, EG, D], bf16, tag="tn")
                    else:
                        ot = epi.tile([128, EG, DOUT], f32, tag="o2")
                    for a in range(EG):
                        zs = z[:, a, :]
                        nc.vector.tensor_scalar_mul(
                            zs, zs, dinv_blk_t[:, g * EG + a:g * EG + a + 1])
                        ztp = psum_e.tile([D, 128], f32, tag="ztp")
                        nc.tensor.transpose(ztp[:], zs, ident[:])
                        zts = epi.tile([D, 128], f32, tag="zts")
                        nc.vector.tensor_copy(zts[:], ztp[:])
                        if layer < 2:
                            op = psum_e.tile([128, D], f32, tag="op")
                            nc.tensor.matmul(op[:], zts[:],
                                             (w0t, w1t, w2t)[layer][:])
                            h = epi.tile([128, D], f32, tag="h")
                            nc.vector.tensor_add(
                                h[:], op[:],
                                b01t[:, layer * D:(layer + 1) * D])
                            hl = epi.tile([128, D], f32, tag="hl")
                            nc.scalar.mul(hl[:], h[:], NEG_SLOPE)
                            nc.vector.tensor_max(hl[:], hl[:], h[:])
                            nc.vector.tensor_scalar_mul(
                                ot[:, a, :], hl[:],
                                dinv_blk_t[:, g * EG + a:g * EG + a + 1])
                        else:
                            op = psum_e.tile([128, DOUT], f32, tag="op")
                            nc.tensor.matmul(op[:], zts[:], w2t[:])
                            nc.vector.tensor_add(ot[:, a, :], op[:], b2t[:])
                    if layer < 2:
                        nc.sync.dma_start(
                            cc_in[sj][sr0:sr1, :].rearrange(
                                "(p a) c -> p a c", p=128), ot[:])
                    else:
                        nc.sync.dma_start(
                            outr[r0:r1, :].rearrange("(p a) c -> p a c",
                                                     p=128), ot[:])

                if layer < 2 and not _SKIP_CC:
                    # Tile tracks the collective/copy tensor deps; Pool
                    # program order pipelines: each seg's copy + gathers of
                    # the next layer run while later AG pieces are on the
                    # wire. Per-seg cc_in tensors let AG-j start as soon as
                    # its own epilogue groups have stored.
                    for j in range(NSEG):
                        nc.gpsimd.collective_compute(
                            "AllGather", mybir.AluOpType.bypass,
                            ins=[cc_in[j][:, :]], outs=[cc_out[j][:, :]],
                            replica_groups=[list(range(W))])

    nc.compile()
    return nc


_CACHE = {}


def _schedule(cfg, edge_index):
    """Preprocess edges and build the smat layout shared by all layers."""
    dinv, sides = _preprocess(cfg, edge_index)
    degs = sorted({blk['d'] for sched, _, _ in sides
                   for blk in sched['blocks']})
    z_degs = sorted({blk['d'] for sched, _, _ in sides
                     for blk in sched['blocks']
                     if any(mm['k0'] == 96 for mm in blk['matmuls'])})
    s_offsets, off = {}, 0
    for d in degs:
        s_offsets[d] = off
        off += 128 // d
    # [block | zeros32 | block]: lev2 (k0=64) reads [0:spp+32] writing zeros
    # over the lev3 range; lev3 (k0=96) reads [spp:spp+32+spp] accumulating
    z_offsets = {}
    for d in z_degs:
        z_offsets[d] = off
        off += 2 * (128 // d) + 32
    s_total = max(16, ((off + 15) // 16) * 16)
    smat = np.zeros((128, s_total), np.float32)
    for d in degs:
        spp = 128 // d
        for t in range(spp * d):
            smat[t, s_offsets[d] + t // d] = 1.0
    for d in z_degs:
        spp = 128 // d
        for t in range(spp * d):
            smat[t, z_offsets[d] + t // d] = 1.0
            smat[t, z_offsets[d] + spp + 32 + t // d] = 1.0
    import ml_dtypes
    smat = smat.astype(ml_dtypes.bfloat16)
    return dinv, sides, s_offsets, z_offsets, s_total, smat


def _get_program(key, cfg, edge_index):
    if key in _CACHE:
        return _CACHE[key]
    dinv, sides, s_offsets, z_offsets, s_total, smat = _schedule(
        cfg, edge_index)
    nc = _build(cfg, sides, s_offsets, z_offsets, s_total)
    _CACHE[key] = (nc, dinv, sides, smat)
    return _CACHE[key]


def kernel(x, edge_index, W0, b0, W1, b1, W2, b2, _cfg=None, _sim=False):
    import ml_dtypes
    x = np.asarray(x, np.float32)
    edge_index = np.asarray(edge_index)
    N, D = x.shape
    DOUT = np.asarray(W2).shape[1]
    cfg = _cfg or _Cfg(N, D, DOUT)
    nc, dinv, sides, smat = _get_program(
        (N, edge_index.shape[1]), cfg, edge_index)

    BP, BLK, Wc = cfg.BP, cfg.BLK, cfg.W
    NTILES = cfg.NTILES

    xs = (x * dinv[:, None]).astype(ml_dtypes.bfloat16)
    tbl0s = [np.zeros((Wc * cfg.SEGR[j], 128), ml_dtypes.bfloat16)
             for j in range(cfg.NSEG)]
    t0b = np.zeros((Wc * BP, D), ml_dtypes.bfloat16)
    for s in range(Wc):
        lo, hi = s * BLK, min((s + 1) * BLK, N)
        n = hi - lo
        for j in range(cfg.NSEG):
            r0, r1 = cfg.RB[j], min(cfg.RB[j + 1], n)
            if r1 > r0:
                sr = cfg.SEGR[j]
                tbl0s[j][s * sr:s * sr + (r1 - r0), 0:D] = xs[lo + r0:lo + r1]
        t0b[s * BP:s * BP + n] = xs[lo:hi]

    b01 = np.zeros((128, 2 * D), np.float32)
    b01[:, :D] = np.asarray(b0, np.float32)[None, :]
    b01[:, D:] = np.asarray(b1, np.float32)[None, :]
    b2t = np.tile(np.asarray(b2, np.float32)[None, :], (128, 1))

    in_maps = []
    for r in range(Wc):
        lo, hi = r * BLK, min((r + 1) * BLK, N)
        db = np.zeros(BP, np.float32)
        db[:hi - lo] = dinv[lo:hi]
        EG = cfg.EG
        # [p, g*EG+a] = db[g*EG*128 + p*EG + a]
        dinv_blk = np.ascontiguousarray(
            db.reshape(NTILES // EG, 128, EG).transpose(1, 0, 2)
            .reshape(128, NTILES))
        im = dict(
            t0_blk=np.ascontiguousarray(t0b[r * BP:(r + 1) * BP]),
            dinv_blk=dinv_blk.astype(np.float32),
            w0=np.asarray(W0, np.float32), w1=np.asarray(W1, np.float32),
            w2=np.asarray(W2, np.float32), b01=b01, b2b=b2t,
            smat=smat,
        )
        for j in range(cfg.NSEG):
            im[f"tbl0s{j}"] = tbl0s[j]
        for c in range(len(cfg.CLS)):
            if sides[c][0]['ntok']:
                im[f"gidx{c}"] = _wrap16(sides[c][1][r])
            if sides[c][0]['ncols']:
                im[f"sidx{c}"] = _wrap16(sides[c][2][r])
        in_maps.append(im)

    if _sim:
        from concourse import bass_interp
        sim = bass_interp.MultiCoreSim(
            nc, Wc, require_finite=False, require_nnan=False)
        for r in range(Wc):
            for k, v in in_maps[r].items():
                sim.cores[r].tensor(k)[:] = v
            sim.cores[r].mem_tensor("outr")[:] = 0
        sim.simulate()
        results = [np.array(sim.cores[r].mem_tensor("outr")).reshape(BP, DOUT)
                   for r in range(Wc)]
    else:
        from concourse.bass_utils import run_bass_kernel_spmd
        res = run_bass_kernel_spmd(nc, in_maps, list(range(Wc)))
        results = [res.results[r]["outr"] for r in range(Wc)]

    out = np.zeros((N, DOUT), np.float32)
    for r in range(Wc):
        lo, hi = r * BLK, min((r + 1) * BLK, N)
        out[lo:hi] = results[r][:hi - lo]
    return out
